# Initial kernel scaffold
#
"""AttentionPooling Bass kernel for 8 TRN2 NeuronCores.

Problem: x [262144, 1024] f32, bags of 128 consecutive rows (2048 bags).
  scores = (tanh(x @ W1 + b1) @ W2 + b2)[:, 0]        per-row MLP score
  w      = softmax(scores) within each bag
  out[b] = sum_i w[i] * x[i]  over the bag's rows  -> [2048, 1024] f32

Sharding: data-parallel over bags; core c gets bags [c*256, (c+1)*256).
Weights replicated. No cross-core communication. b2 is dropped (uniform
shift inside each bag's softmax — mathematically a no-op for the output).

Per-core dataflow (bf16 matmul precision, fp32 accumulation):
  phase 1 (per bag = one 128-row tile): HWDGE-load x rows f32, cast to
    bf16 on VectorE; PE-transpose the 8 [128,128] chunks (the PE
    contracts along partitions, so x^T is required); 16 accumulating
    bf16 matmuls against resident W1 -> S [128,1024] in PSUM; tanh on
    ScalarE; fused multiply+reduce against replicated W2 on VectorE ->
    per-row scores, written into a [128 rows, 8 bags] group tile.
  softmax (per 8-bag group): PE-transpose scores -> [bag, row];
    reduce_max (negated), exp with per-partition bias + fused sum,
    reciprocal, scale -> weights [bag, row]; PE-transpose back ->
    per-bag weight columns [row, bag].
  phase 2 (per 4 bags): M=1 matmuls w_bag^T @ x_bag (x bf16 tiles kept
    resident from phase 1) at 4 PSUM col-group positions, ScalarE copy
    to SBUF, DMA rows out.
  Softmax+phase 2 of group g are emitted after phase 1 of group g+1 so
  the PE never stalls on the softmax's cross-engine chain.
"""

import sys

if "/opt/trn_rl_repo" not in sys.path:
    sys.path.insert(0, "/opt/trn_rl_repo")

import numpy as np

import concourse.bass as bass
import concourse.bacc as bacc
import concourse.mybir as mybir
import concourse.tile as tile
from concourse.bass_utils import run_bass_kernel_spmd
from concourse.masks import make_identity

F32 = mybir.dt.float32
BF16 = mybir.dt.bfloat16
AF = mybir.ActivationFunctionType
ALU = mybir.AluOpType

N_CORES = 8
BAG = 128
D = 1024
H = 1024
DC = D // 128  # contraction chunks
GROUP = 8      # bags per softmax group
WG = 4         # bags per weighted-sum subgroup (PSUM col-group packing)

# set by test.py for profiling; the grading harness leaves these alone
TRACE = False
LAST_EXEC_NS = None
LAST_PROFILE = None

_cache = {}


def _build(bags_core: int, with_b1: bool, n_cores: int = N_CORES, stage: int = 3):
    """Build the per-core Bass module. All cores run the same NEFF."""
    assert bags_core % GROUP == 0 and GROUP % WG == 0
    rows_core = bags_core * BAG
    n_groups = bags_core // GROUP

    nc = bacc.Bacc("TRN2", target_bir_lowering=False, debug=False,
                   num_devices=n_cores)
    x_h = nc.declare_dram_parameter("x", [rows_core, D], F32, isOutput=False)
    w1_h = nc.declare_dram_parameter("w1", [D, H], F32, isOutput=False)
    w2_h = nc.declare_dram_parameter("w2", [1, H], F32, isOutput=False)
    b1_h = nc.declare_dram_parameter("b1", [1, H], F32, isOutput=False)
    out_h = nc.declare_dram_parameter("out", [bags_core, D], F32, isOutput=True)

    with tile.TileContext(nc) as tc:
        with (
            tc.tile_pool(name="const", bufs=1) as const_pool,
            tc.tile_pool(name="xstage", bufs=4) as xs_pool,
            tc.tile_pool(name="xb", bufs=2 * GROUP + 4) as xb_pool,
            tc.tile_pool(name="xt", bufs=3) as xt_pool,
            tc.tile_pool(name="tanh", bufs=2) as t_pool,
            tc.tile_pool(name="dump", bufs=1) as dump_pool,
            tc.tile_pool(name="scores", bufs=2) as sc_pool,
            tc.tile_pool(name="soft", bufs=2) as soft_pool,
            tc.tile_pool(name="ystage", bufs=2) as y_pool,
            tc.tile_pool(name="ps_xt", bufs=2, space="PSUM") as ps_xt_pool,
            tc.tile_pool(name="ps_s", bufs=2, space="PSUM") as ps_s_pool,
            tc.tile_pool(name="ps_y", bufs=2, space="PSUM") as ps_y_pool,
            tc.tile_pool(name="ps_sm", bufs=1, space="PSUM") as ps_sm_pool,
        ):
            # ---- constants / weights (resident) ----
            ident_b = const_pool.tile([128, 128], BF16)
            make_identity(nc, ident_b)
            ident_f = const_pool.tile([128, 128], F32)
            make_identity(nc, ident_f)

            w1_sb = const_pool.tile([128, DC, H], BF16)
            for c in range(DC):
                nc.gpsimd.dma_start(out=w1_sb[:, c, :],
                                    in_=w1_h[c * 128:(c + 1) * 128, :])

            w2_row = const_pool.tile([1, H], BF16)
            nc.gpsimd.dma_start(out=w2_row[:, :], in_=w2_h[:, :])
            ones_row = const_pool.tile([1, 128], BF16)
            nc.any.memset(ones_row[:, :], 1.0)
            # replicate W2 across partitions: ones[1,128].T @ w2_row[1,512]
            w2_rep = const_pool.tile([128, H], BF16)
            for j in range(2):
                ps = ps_sm_pool.tile([128, 512], F32, tag="smps")
                nc.tensor.matmul(ps[:, :], lhsT=ones_row[:, :],
                                 rhs=w2_row[:, 512 * j:512 * (j + 1)],
                                 start=True, stop=True)
                nc.vector.tensor_copy(w2_rep[:, 512 * j:512 * (j + 1)], ps[:, :])

            if with_b1:
                b1_row = const_pool.tile([1, H], BF16)
                nc.gpsimd.dma_start(out=b1_row[:, :], in_=b1_h[:, :])

            def phase1(g):
                """Scores for the group's bags; returns (sc_tile, x tiles)."""
                sc_tile = sc_pool.tile([128, GROUP], F32)
                xbs = []
                for n in range(GROUP):
                    bag = g * GROUP + n
                    x_s = xs_pool.tile([128, D], F32)
                    nc.sync.dma_start(out=x_s[:, :],
                                      in_=x_h[bag * BAG:(bag + 1) * BAG, :])
                    x_b = xb_pool.tile([128, D], BF16)
                    nc.vector.tensor_copy(x_b[:, :], x_s[:, :])
                    xbs.append(x_b)

                    ps_xt = ps_xt_pool.tile([128, DC, 128], BF16)
                    for c in range(DC):
                        nc.tensor.transpose(ps_xt[:, c, :],
                                            x_b[:, c * 128:(c + 1) * 128],
                                            ident_b[:, :])
                    xt_sb = xt_pool.tile([128, DC, 128], BF16)
                    nc.vector.tensor_copy(xt_sb[:, :, :], ps_xt[:, :, :])

                    t_t = t_pool.tile([128, H], BF16)
                    for j in range(2):
                        ps_s = ps_s_pool.tile([128, 512], F32)
                        for c in range(DC):
                            nc.tensor.matmul(ps_s[:, :],
                                             lhsT=xt_sb[:, c, :],
                                             rhs=w1_sb[:, c, 512 * j:512 * (j + 1)],
                                             start=(c == 0),
                                             stop=(c == DC - 1 and not with_b1))
                        if with_b1:
                            nc.tensor.matmul(ps_s[:, :], lhsT=ones_row[:, :],
                                             rhs=b1_row[:, 512 * j:512 * (j + 1)],
                                             start=False, stop=True)
                        nc.scalar.activation(t_t[:, 512 * j:512 * (j + 1)],
                                             ps_s[:, :], AF.Tanh)

                    dump = dump_pool.tile([128, H], BF16)
                    nc.vector.tensor_mul(dump[:, :], t_t[:, :], w2_rep[:, :])
                    nc.vector.reduce_sum(sc_tile[:, n:n + 1], dump[:, :],
                                         axis=mybir.AxisListType.X)
                return sc_tile, xbs

            def softmax_wsum(g, sc_tile, xbs):
                if stage < 2:
                    # bisection stub: dump raw scores
                    for n in range(GROUP):
                        nc.sync.dma_start(out=out_h[g * GROUP + n, 0:128],
                                          in_=sc_tile[:, n:n + 1])
                    return
                # batched softmax over the group's bags
                ps_sc = ps_sm_pool.tile([GROUP, 128], F32, tag="smps")
                nc.tensor.transpose(ps_sc[:, :], sc_tile[:, :], ident_f[:, :])
                sct = soft_pool.tile([GROUP, 128], F32)
                nc.vector.tensor_copy(sct[:, :], ps_sc[:, :])
                neg_mx = soft_pool.tile([GROUP, 1], F32)
                nc.vector.tensor_reduce(neg_mx[:, :], sct[:, :],
                                        axis=mybir.AxisListType.X,
                                        op=ALU.max, negate=True)
                e_t = soft_pool.tile([GROUP, 128], F32)
                sum_t = soft_pool.tile([GROUP, 1], F32)
                nc.scalar.activation(e_t[:, :], sct[:, :], AF.Exp,
                                     bias=neg_mx[:, :], scale=1.0,
                                     accum_out=sum_t[:, :])
                rcp = soft_pool.tile([GROUP, 1], F32)
                nc.vector.reciprocal(rcp[:, :], sum_t[:, :])
                wt = soft_pool.tile([GROUP, 128], BF16)
                nc.vector.tensor_scalar_mul(wt[:, :], e_t[:, :], rcp[:, :])
                ps_wc = ps_sm_pool.tile([128, GROUP], BF16, tag="smps")
                nc.tensor.transpose(ps_wc[:, :], wt[:, :],
                                    ident_b[:GROUP, :GROUP])
                w_cols = soft_pool.tile([128, GROUP], BF16)
                nc.vector.tensor_copy(w_cols[:, :], ps_wc[:, :])
                if stage == 2:
                    nc.gpsimd.dma_start(out=out_h[g * GROUP:(g + 1) * GROUP, 0:128],
                                        in_=wt[:, :])
                    return

                # weighted sums, WG bags at a time via PSUM col-groups
                for q in range(GROUP // WG):
                    ys = y_pool.tile([128, D], F32)
                    if stage == 25:
                        nc.vector.tensor_copy(ys[:, :], xbs[q][:, :])
                    else:
                        for j in range(2):
                            ps_y = ps_y_pool.tile([128, 512], F32)
                            for v in range(WG):
                                b = q * WG + v
                                nc.tensor.matmul(ps_y[32 * v:32 * v + 1, :],
                                                 lhsT=w_cols[:, b:b + 1],
                                                 rhs=xbs[b][:, 512 * j:512 * (j + 1)],
                                                 start=True, stop=True,
                                                 tile_position=(0, 32 * v))
                            # NOTE: nc.scalar.copy here hangs the device
                            # (ScalarE read of the partially-written PSUM
                            # bank); VectorE is fine.
                            nc.vector.tensor_copy(ys[:, 512 * j:512 * (j + 1)],
                                                  ps_y[:, :])
                    for v in range(WG):
                        bag = g * GROUP + q * WG + v
                        nc.sync.dma_start(out=out_h[bag:bag + 1, :],
                                          in_=ys[32 * v:32 * v + 1, :])

            prev = None
            for g in range(n_groups):
                cur = phase1(g)
                if prev is not None:
                    softmax_wsum(g - 1, *prev)
                prev = cur
            softmax_wsum(n_groups - 1, *prev)

    nc.finalize()
    return nc


def _numpy_fallback(x, W1, b1, W2, b2, bag_sizes):
    seg_ends = np.cumsum(bag_sizes)
    seg_starts = seg_ends - bag_sizes
    scores = (np.tanh(x @ W1 + b1) @ W2 + b2)[:, 0]
    out = np.zeros((bag_sizes.shape[0], x.shape[1]), dtype=x.dtype)
    for i, (s, e) in enumerate(zip(seg_starts, seg_ends)):
        sc = scores[s:e]
        w = np.exp(sc - sc.max())
        w /= w.sum()
        out[i] = w @ x[s:e]
    return out


def kernel(x, W1, b1, W2, b2, bag_sizes):
    x = np.ascontiguousarray(np.asarray(x, dtype=np.float32))
    W1 = np.ascontiguousarray(np.asarray(W1, dtype=np.float32))
    b1 = np.asarray(b1, dtype=np.float32)
    W2 = np.asarray(W2, dtype=np.float32)
    b2 = np.asarray(b2, dtype=np.float32)
    bag_sizes = np.asarray(bag_sizes)

    n_bags = bag_sizes.shape[0]
    if not (np.all(bag_sizes == BAG) and x.shape[0] == n_bags * BAG
            and x.shape[1] == D and n_bags % (N_CORES * GROUP) == 0):
        return _numpy_fallback(x, W1, b1, W2, b2, bag_sizes)

    bags_core = n_bags // N_CORES
    rows_core = bags_core * BAG
    with_b1 = bool(np.any(b1))

    key = (bags_core, with_b1)
    if key not in _cache:
        _cache[key] = _build(bags_core, with_b1)
    nc = _cache[key]

    w2_row = np.ascontiguousarray(W2.reshape(1, H))
    b1_row = np.ascontiguousarray(b1.reshape(1, H))
    in_maps = []
    for c in range(N_CORES):
        in_maps.append({
            "x": x[c * rows_core:(c + 1) * rows_core],
            "w1": W1,
            "w2": w2_row,
            "b1": b1_row,
        })

    res = run_bass_kernel_spmd(nc, in_maps, core_ids=list(range(N_CORES)),
                               trace=TRACE)
    global LAST_EXEC_NS, LAST_PROFILE
    LAST_EXEC_NS = res.exec_time_ns
    LAST_PROFILE = res.profile_json

    return np.concatenate([res.results[c]["out"] for c in range(N_CORES)], axis=0)



# revision 4
# speedup vs baseline: 1.4145x; 1.4145x over previous
"""AttentionPooling Bass kernel for 8 TRN2 NeuronCores.

Problem: x [262144, 1024] f32, bags of 128 consecutive rows (2048 bags).
  scores = (tanh(x @ W1 + b1) @ W2 + b2)[:, 0]        per-row MLP score
  w      = softmax(scores) within each bag
  out[b] = sum_i w[i] * x[i]  over the bag's rows  -> [2048, 1024] f32

Sharding: data-parallel over bags; core c gets bags [c*256, (c+1)*256).
Weights replicated. No cross-core communication. b2 is dropped (uniform
shift inside each bag's softmax — mathematically a no-op for the output).

Per-core dataflow (bf16 matmul precision, fp32 accumulation):
  phase 1 (per bag = one 128-row tile): HWDGE-load x rows f32, cast to
    bf16 on VectorE; PE-transpose the 8 [128,128] chunks (the PE
    contracts along partitions, so x^T is required); 16 accumulating
    bf16 matmuls against resident W1 -> S [128,1024] in PSUM; tanh on
    ScalarE; fused multiply+reduce against replicated W2 on VectorE ->
    per-row scores, written into a [128 rows, 8 bags] group tile.
  softmax (per 8-bag group): PE-transpose scores -> [bag, row];
    reduce_max (negated), exp with per-partition bias + fused sum,
    reciprocal, scale -> weights [bag, row]; PE-transpose back ->
    per-bag weight columns [row, bag].
  phase 2 (per 4 bags): M=1 matmuls w_bag^T @ x_bag (x bf16 tiles kept
    resident from phase 1) at 4 PSUM col-group positions, ScalarE copy
    to SBUF, DMA rows out.
  Softmax+phase 2 of group g are emitted after phase 1 of group g+1 so
  the PE never stalls on the softmax's cross-engine chain.
"""

import sys

if "/opt/trn_rl_repo" not in sys.path:
    sys.path.insert(0, "/opt/trn_rl_repo")

import numpy as np

import concourse.bass as bass
import concourse.bacc as bacc
import concourse.mybir as mybir
import concourse.tile as tile
from concourse.bass_utils import run_bass_kernel_spmd
from concourse.masks import make_identity

F32 = mybir.dt.float32
BF16 = mybir.dt.bfloat16
AF = mybir.ActivationFunctionType
ALU = mybir.AluOpType

N_CORES = 8
BAG = 128
D = 1024
H = 1024
DC = D // 128  # contraction chunks
GROUP = 8      # bags per softmax group
WG = 4         # bags per weighted-sum subgroup (PSUM col-group packing)

# set by test.py for profiling; the grading harness leaves these alone
TRACE = False
LAST_EXEC_NS = None
LAST_PROFILE = None
LAST_NC = None

_cache = {}


def _build(bags_core: int, with_b1: bool, n_cores: int = N_CORES, stage: int = 3):
    """Build the per-core Bass module. All cores run the same NEFF."""
    assert bags_core % GROUP == 0 and GROUP % WG == 0
    rows_core = bags_core * BAG
    n_groups = bags_core // GROUP

    nc = bacc.Bacc("TRN2", target_bir_lowering=False, debug=False,
                   num_devices=n_cores)
    x_h = nc.declare_dram_parameter("x", [rows_core, D], F32, isOutput=False)
    w1_h = nc.declare_dram_parameter("w1", [D, H], F32, isOutput=False)
    w2_h = nc.declare_dram_parameter("w2", [1, H], F32, isOutput=False)
    b1_h = nc.declare_dram_parameter("b1", [1, H], F32, isOutput=False)
    out_h = nc.declare_dram_parameter("out", [bags_core, D], F32, isOutput=True)

    with tile.TileContext(nc) as tc:
        with (
            tc.tile_pool(name="const", bufs=1) as const_pool,
            tc.tile_pool(name="xstage", bufs=4) as xs_pool,
            tc.tile_pool(name="xb", bufs=2 * GROUP + 4) as xb_pool,
            tc.tile_pool(name="xt", bufs=3) as xt_pool,
            tc.tile_pool(name="tanh", bufs=2) as t_pool,
            tc.tile_pool(name="dump", bufs=1) as dump_pool,
            tc.tile_pool(name="scores", bufs=2) as sc_pool,
            tc.tile_pool(name="soft", bufs=2) as soft_pool,
            tc.tile_pool(name="ystage", bufs=2) as y_pool,
            tc.tile_pool(name="ps_xt", bufs=2, space="PSUM") as ps_xt_pool,
            tc.tile_pool(name="ps_s", bufs=2, space="PSUM") as ps_s_pool,
            tc.tile_pool(name="ps_y", bufs=2, space="PSUM") as ps_y_pool,
            tc.tile_pool(name="ps_sm", bufs=1, space="PSUM") as ps_sm_pool,
        ):
            # ---- constants / weights (resident) ----
            ident_b = const_pool.tile([128, 128], BF16)
            make_identity(nc, ident_b)
            ident_f = const_pool.tile([128, 128], F32)
            make_identity(nc, ident_f)

            w1_sb = const_pool.tile([128, DC, H], BF16)
            for c in range(DC):
                nc.gpsimd.dma_start(out=w1_sb[:, c, :],
                                    in_=w1_h[c * 128:(c + 1) * 128, :])

            w2_row = const_pool.tile([1, H], BF16)
            nc.gpsimd.dma_start(out=w2_row[:, :], in_=w2_h[:, :])
            ones_row = const_pool.tile([1, 128], BF16)
            nc.any.memset(ones_row[:, :], 1.0)
            # replicate W2 across partitions: ones[1,128].T @ w2_row[1,512]
            w2_rep = const_pool.tile([128, H], BF16)
            for j in range(2):
                ps = ps_sm_pool.tile([128, 512], F32, tag="smps")
                nc.tensor.matmul(ps[:, :], lhsT=ones_row[:, :],
                                 rhs=w2_row[:, 512 * j:512 * (j + 1)],
                                 start=True, stop=True)
                nc.vector.tensor_copy(w2_rep[:, 512 * j:512 * (j + 1)], ps[:, :])

            if with_b1:
                b1_row = const_pool.tile([1, H], BF16)
                nc.gpsimd.dma_start(out=b1_row[:, :], in_=b1_h[:, :])

            def phase1(g):
                """Scores for the group's bags; returns (sc_tile, x tiles)."""
                sc_tile = sc_pool.tile([128, GROUP], F32)
                xbs = []
                for n in range(GROUP):
                    bag = g * GROUP + n
                    x_s = xs_pool.tile([128, D], F32)
                    nc.sync.dma_start(out=x_s[:, :],
                                      in_=x_h[bag * BAG:(bag + 1) * BAG, :])
                    x_b = xb_pool.tile([128, D], BF16)
                    nc.vector.tensor_copy(x_b[:, :], x_s[:, :])
                    xbs.append(x_b)

                    ps_xt = ps_xt_pool.tile([128, DC, 128], BF16)
                    for c in range(DC):
                        nc.tensor.transpose(ps_xt[:, c, :],
                                            x_b[:, c * 128:(c + 1) * 128],
                                            ident_b[:, :])
                    xt_sb = xt_pool.tile([128, DC, 128], BF16)
                    nc.vector.tensor_copy(xt_sb[:, :, :], ps_xt[:, :, :])

                    t_t = t_pool.tile([128, H], BF16)
                    for j in range(2):
                        ps_s = ps_s_pool.tile([128, 512], F32)
                        for c in range(DC):
                            nc.tensor.matmul(ps_s[:, :],
                                             lhsT=xt_sb[:, c, :],
                                             rhs=w1_sb[:, c, 512 * j:512 * (j + 1)],
                                             start=(c == 0),
                                             stop=(c == DC - 1 and not with_b1))
                        if with_b1:
                            nc.tensor.matmul(ps_s[:, :], lhsT=ones_row[:, :],
                                             rhs=b1_row[:, 512 * j:512 * (j + 1)],
                                             start=False, stop=True)
                        nc.scalar.activation(t_t[:, 512 * j:512 * (j + 1)],
                                             ps_s[:, :], AF.Tanh)

                    dump = dump_pool.tile([128, H], BF16)
                    nc.vector.tensor_mul(dump[:, :], t_t[:, :], w2_rep[:, :])
                    nc.vector.reduce_sum(sc_tile[:, n:n + 1], dump[:, :],
                                         axis=mybir.AxisListType.X)
                return sc_tile, xbs

            def softmax_wsum(g, sc_tile, xbs):
                if stage < 2:
                    # bisection stub: dump raw scores
                    for n in range(GROUP):
                        nc.sync.dma_start(out=out_h[g * GROUP + n, 0:128],
                                          in_=sc_tile[:, n:n + 1])
                    return
                # batched softmax over the group's bags
                ps_sc = ps_sm_pool.tile([GROUP, 128], F32, tag="smps")
                nc.tensor.transpose(ps_sc[:, :], sc_tile[:, :], ident_f[:, :])
                sct = soft_pool.tile([GROUP, 128], F32)
                nc.vector.tensor_copy(sct[:, :], ps_sc[:, :])
                neg_mx = soft_pool.tile([GROUP, 1], F32)
                nc.vector.tensor_reduce(neg_mx[:, :], sct[:, :],
                                        axis=mybir.AxisListType.X,
                                        op=ALU.max, negate=True)
                e_t = soft_pool.tile([GROUP, 128], F32)
                sum_t = soft_pool.tile([GROUP, 1], F32)
                nc.scalar.activation(e_t[:, :], sct[:, :], AF.Exp,
                                     bias=neg_mx[:, :], scale=1.0,
                                     accum_out=sum_t[:, :])
                rcp = soft_pool.tile([GROUP, 1], F32)
                nc.vector.reciprocal(rcp[:, :], sum_t[:, :])
                wt = soft_pool.tile([GROUP, 128], BF16)
                nc.vector.tensor_scalar_mul(wt[:, :], e_t[:, :], rcp[:, :])
                ps_wc = ps_sm_pool.tile([128, GROUP], BF16, tag="smps")
                nc.tensor.transpose(ps_wc[:, :], wt[:, :],
                                    ident_b[:GROUP, :GROUP])
                w_cols = soft_pool.tile([128, GROUP], BF16)
                nc.vector.tensor_copy(w_cols[:, :], ps_wc[:, :])
                if stage == 2:
                    nc.gpsimd.dma_start(out=out_h[g * GROUP:(g + 1) * GROUP, 0:128],
                                        in_=wt[:, :])
                    return

                # weighted sums, WG bags at a time via PSUM col-groups
                for q in range(GROUP // WG):
                    ys = y_pool.tile([128, D], F32)
                    if stage == 25:
                        nc.vector.tensor_copy(ys[:, :], xbs[q][:, :])
                    else:
                        for j in range(2):
                            ps_y = ps_y_pool.tile([128, 512], F32)
                            for v in range(WG):
                                b = q * WG + v
                                nc.tensor.matmul(ps_y[32 * v:32 * v + 1, :],
                                                 lhsT=w_cols[:, b:b + 1],
                                                 rhs=xbs[b][:, 512 * j:512 * (j + 1)],
                                                 start=True, stop=True,
                                                 tile_position=(0, 32 * v))
                            # NOTE: nc.scalar.copy here hangs the device
                            # (ScalarE read of the partially-written PSUM
                            # bank); VectorE is fine.
                            nc.vector.tensor_copy(ys[:, 512 * j:512 * (j + 1)],
                                                  ps_y[:, :])
                    for v in range(WG):
                        bag = g * GROUP + q * WG + v
                        nc.sync.dma_start(out=out_h[bag:bag + 1, :],
                                          in_=ys[32 * v:32 * v + 1, :])

            prev = None
            for g in range(n_groups):
                cur = phase1(g)
                if prev is not None:
                    softmax_wsum(g - 1, *prev)
                prev = cur
            softmax_wsum(n_groups - 1, *prev)

    nc.finalize()
    return nc


def _numpy_fallback(x, W1, b1, W2, b2, bag_sizes):
    seg_ends = np.cumsum(bag_sizes)
    seg_starts = seg_ends - bag_sizes
    scores = (np.tanh(x @ W1 + b1) @ W2 + b2)[:, 0]
    out = np.zeros((bag_sizes.shape[0], x.shape[1]), dtype=x.dtype)
    for i, (s, e) in enumerate(zip(seg_starts, seg_ends)):
        sc = scores[s:e]
        w = np.exp(sc - sc.max())
        w /= w.sum()
        out[i] = w @ x[s:e]
    return out


def kernel(x, W1, b1, W2, b2, bag_sizes):
    x = np.ascontiguousarray(np.asarray(x, dtype=np.float32))
    W1 = np.ascontiguousarray(np.asarray(W1, dtype=np.float32))
    b1 = np.asarray(b1, dtype=np.float32)
    W2 = np.asarray(W2, dtype=np.float32)
    b2 = np.asarray(b2, dtype=np.float32)
    bag_sizes = np.asarray(bag_sizes)

    n_bags = bag_sizes.shape[0]
    if not (np.all(bag_sizes == BAG) and x.shape[0] == n_bags * BAG
            and x.shape[1] == D and n_bags % (N_CORES * GROUP) == 0):
        return _numpy_fallback(x, W1, b1, W2, b2, bag_sizes)

    bags_core = n_bags // N_CORES
    rows_core = bags_core * BAG
    with_b1 = bool(np.any(b1))

    key = (bags_core, with_b1)
    if key not in _cache:
        _cache[key] = _build(bags_core, with_b1)
    nc = _cache[key]
    global LAST_NC
    LAST_NC = nc

    w2_row = np.ascontiguousarray(W2.reshape(1, H))
    b1_row = np.ascontiguousarray(b1.reshape(1, H))
    in_maps = []
    for c in range(N_CORES):
        in_maps.append({
            "x": x[c * rows_core:(c + 1) * rows_core],
            "w1": W1,
            "w2": w2_row,
            "b1": b1_row,
        })

    res = run_bass_kernel_spmd(nc, in_maps, core_ids=list(range(N_CORES)),
                               trace=False)
    global LAST_EXEC_NS, LAST_PROFILE
    LAST_EXEC_NS = res.exec_time_ns
    LAST_PROFILE = res.profile_json

    return np.concatenate([res.results[c]["out"] for c in range(N_CORES)], axis=0)

